# revision 1
# baseline (speedup 1.0000x reference)
"""BandLinear kernel for 8 TRN2 NeuronCores.

out[n, o] = sum_i x[n, i] * (weight * mask)[o, i] + bias[o]
with a +-8 band mask, x: [16384, 4096] f32.

Strategy (data-parallel over tokens, 2048 tokens/core):
 - Host pre-transposes each core's x shard into in-feature-block tiles
   xt[k, p, n] = x[n, 128k + p]  (k: 32 in-blocks, p: partition, n: token)
 - Weights are packed per out-block j as 3 stationary couplings
   ws[j, p, 128d + m] = (weight*mask)[128j + m, 128(j-1+d) + p]
 - On device, for each out-block j, psum[o_local, n] accumulates 2-3
   matmuls (stationary = coupling weights, moving = x tiles, N=512
   token chunks, one PSUM bank per chunk). Bias is per-partition and is
   fused into the PSUM->SBUF drain (ScalarE Identity / VectorE
   tensor_scalar). Device writes out^T [4096, 2048]; host un-transposes.
 - x and out^T blocks are shipped as k/j PAIRS ([128, 2*2048] tiles with
   fully contiguous per-partition DMA lines) to halve DMA instruction
   count and maximize transfer efficiency; the host packs/unpacks.
"""

import os
import sys

for _p in ("/opt/trn_rl_repo", "/root/.axon_site/_ro/trn_rl_repo"):
    if os.path.isdir(_p) and _p not in sys.path:
        sys.path.append(_p)

import numpy as np
import ml_dtypes

import concourse.bacc as bacc
import concourse.mybir as mybir
from concourse.bass_utils import run_bass_kernel_spmd
from concourse.tile import TileContext

N_CORES = 8
N_TOK = 16384
NF = 4096
BAND = 8
TPC = N_TOK // N_CORES          # tokens per core (2048)
KB = NF // 128                  # 32 feature blocks
CC = TPC // 512                 # token chunks of 512 per core (4)

# compute dtype: "float32r" (1 cyc/row, ~2e-4), "bfloat16" (~3e-3, half DMA),
# "float32" (exact, 4 cyc/row)
# Measured on HW (8 cores, full problem; DMA-bound, HBM wall ~358 GB/s/core):
#   bfloat16/bfloat16: ~106 us, absmax-rel err 5.2e-3 (resid_var 6.9e-6)
#   bfloat16/float32:  ~143 us, absmax-rel err 2.9e-3
#   float32r/float32:  ~220 us, absmax-rel err 2.1e-4
COMPUTE_DT = os.environ.get("BAND_COMPUTE_DT", "bfloat16")
# output storage dtype: "float32" or "bfloat16" (host upcasts to f32 either way)
OUT_DT = os.environ.get("BAND_OUT_DT", "bfloat16")

LAST_RESULT = None  # BassKernelResults of the most recent run (for test.py)

_cache = {}


def _np_dt(name):
    return ml_dtypes.bfloat16 if name == "bfloat16" else np.float32


def _build(compute_dt: str, out_dt: str):
    cdt = getattr(mybir.dt, compute_dt)
    odt = getattr(mybir.dt, out_dt)
    f32 = mybir.dt.float32
    nc = bacc.Bacc("TRN2", target_bir_lowering=False, debug=False,
                   num_devices=N_CORES)
    NQ = KB // 2                   # 16 k-pair tiles
    # xq[q, p, kk*TPC + n] = x[n, 128*(2q+kk) + p]
    XQ = nc.dram_tensor("xt", [NQ, 128, 2 * TPC], cdt,
                        kind="ExternalInput").ap()
    WS = nc.dram_tensor("ws", [4, 128, (KB // 4) * 384], cdt,
                        kind="ExternalInput").ap()
    BM = nc.dram_tensor("bm", [128, KB], f32, kind="ExternalInput").ap()
    # ou[u, p, jj*TPC + n] = out^T[128*(2u+jj) + p, n]
    OU = nc.dram_tensor("out", [NQ, 128, 2 * TPC], odt,
                        kind="ExternalOutput").ap()

    ident = mybir.ActivationFunctionType.Identity
    add = mybir.AluOpType.add
    WCH = (KB // 4) * 384          # ws chunk width (8 j's worth)
    xq_bufs = 6 if compute_dt == "bfloat16" else 5
    o_bufs = 3 if out_dt == "bfloat16" else 2

    with TileContext(nc) as tc:
        with (
            tc.tile_pool(name="bp", bufs=1) as bp,
            tc.tile_pool(name="xp", bufs=xq_bufs) as xp,
            tc.tile_pool(name="op", bufs=o_bufs) as op,
            tc.tile_pool(name="pp", bufs=8, space="PSUM") as pp,
        ):
            bias_sb = bp.tile([128, KB], f32)
            ws_sb = bp.tile([128, KB * 384], cdt)

            xq_sb = {}

            def load_xq(q):
                t = xp.tile([128, 2 * TPC], cdt, tag="xq")
                nc.sync.dma_start(out=t[:], in_=XQ[q])
                xq_sb[q] = t

            def x_slice(k, c):
                t = xq_sb[k // 2]
                base = (k % 2) * TPC + 512 * c
                return t[:, base:base + 512]

            # Interleave initial x prefetches with the 4 weight chunks so
            # the first matmuls start early while DMA stays saturated.
            nc.sync.dma_start(out=bias_sb[:], in_=BM[:])
            load_xq(0)
            nc.sync.dma_start(out=ws_sb[:, 0:WCH], in_=WS[0])
            load_xq(1)
            nc.sync.dma_start(out=ws_sb[:, WCH:2 * WCH], in_=WS[1])
            load_xq(2)
            nc.sync.dma_start(out=ws_sb[:, 2 * WCH:3 * WCH], in_=WS[2])
            nc.sync.dma_start(out=ws_sb[:, 3 * WCH:4 * WCH], in_=WS[3])

            oj2 = None
            for j in range(KB):
                if j % 2 == 0 and j // 2 + 3 < NQ:
                    load_xq(j // 2 + 3)
                if j % 2 == 0:
                    oj2 = op.tile([128, 2 * TPC], odt, tag="o")
                ds = [d for d in range(3) if 0 <= j - 1 + d < KB]
                for c in range(CC):
                    p = pp.tile([128, 512], f32, tag="ps")
                    for i, d in enumerate(ds):
                        nc.tensor.matmul(
                            p[:],
                            ws_sb[:, j * 384 + 128 * d:j * 384 + 128 * d + 128],
                            x_slice(j - 1 + d, c),
                            start=(i == 0),
                            stop=(i == len(ds) - 1),
                        )
                    ob = (j % 2) * TPC + 512 * c
                    osl = oj2[:, ob:ob + 512]
                    bsl = bias_sb[:, j:j + 1]
                    if (j + c) % 2 == 0:
                        nc.scalar.activation(osl, p[:], ident, bias=bsl)
                    else:
                        nc.vector.tensor_scalar(osl, p[:], bsl, None, op0=add)
                if j % 2 == 1:
                    nc.sync.dma_start(out=OU[j // 2], in_=oj2[:])
    nc.finalize()
    return nc


def _get_nc(compute_dt, out_dt):
    key = (compute_dt, out_dt)
    if key not in _cache:
        _cache[key] = _build(compute_dt, out_dt)
    return _cache[key]


def kernel(x, weight, bias, mask):
    global LAST_RESULT
    x = np.asarray(x, dtype=np.float32)
    weight = np.asarray(weight, dtype=np.float32)
    bias = np.asarray(bias, dtype=np.float32)
    mask = np.asarray(mask, dtype=np.float32)

    cnp = _np_dt(COMPUTE_DT)
    wm = weight * mask                      # [O, I]

    # ws[j, p, 128d + m] = wm[128j + m, 128(j-1+d) + p]
    ws = np.zeros((KB, 128, 384), dtype=np.float32)
    for j in range(KB):
        for d in range(3):
            jj = j - 1 + d
            if 0 <= jj < KB:
                blk = wm[128 * j:128 * j + 128, 128 * jj:128 * jj + 128]
                ws[j, :, 128 * d:128 * d + 128] = blk.T
    ws = np.ascontiguousarray(
        ws.astype(cnp).reshape(4, KB // 4, 128, 384).transpose(0, 2, 1, 3)
        .reshape(4, 128, (KB // 4) * 384))

    bm = np.ascontiguousarray(bias.reshape(KB, 128).T.astype(np.float32))

    in_maps = []
    for ci in range(N_CORES):
        xs = x[TPC * ci:TPC * (ci + 1)]               # [TPC, NF]
        # xq[q, p, kk*TPC + n] = xs[n, 128*(2q+kk) + p]
        xq = (np.ascontiguousarray(xs.T.astype(cnp))
              .reshape(KB // 2, 2, 128, TPC).transpose(0, 2, 1, 3))
        xq = np.ascontiguousarray(xq).reshape(KB // 2, 128, 2 * TPC)
        in_maps.append({"xt": xq, "ws": ws, "bm": bm})

    nc = _get_nc(COMPUTE_DT, OUT_DT)
    LAST_RESULT = run_bass_kernel_spmd(nc, in_maps, list(range(N_CORES)))

    out = np.empty((N_TOK, NF), dtype=np.float32)
    for ci in range(N_CORES):
        ou = np.asarray(LAST_RESULT.results[ci]["out"], dtype=np.float32)
        ot = (ou.reshape(KB // 2, 128, 2, TPC).transpose(0, 2, 1, 3)
              .reshape(NF, TPC))
        out[TPC * ci:TPC * (ci + 1)] = ot.T
    return out



# revision 3
# speedup vs baseline: 1.0857x; 1.0857x over previous
"""BandLinear kernel for 8 TRN2 NeuronCores — final (overlapped 112-stride tiles, two-bank
PSUM drains, paired 8KB-line output DMAs, co-scheduled read/write
streams, barrier-gap-filling lead loads).

out[n, o] = sum_i x[n, i] * (weight * mask)[o, i] + bias[o]
with a +-8 band mask, x: [16384, 4096] f32.

Strategy (data-parallel over tokens, 2048 tokens/core):
 - Host pre-transposes each core's x shard into OVERLAPPED tiles of 128
   in-feature rows at stride 112: tile_k[p, n] = x[n, 112k - 8 + p]
   (zero-padded off the edges). A 112-row output block k (rows
   112k..112k+111) has its whole +-8 band inside tile k, so it needs
   exactly ONE K=128 matmul per token chunk — no accumulation couplings,
   no PE row-group switches (those flush the PE pipeline and were worth
   ~40us), at the cost of shipping 14% extra x bytes.
 - 37 tiles cover 4096 rows; tiles ride in pairs ([128, 2*2048] DMAs,
   8KB contiguous per-partition lines) + one single tail tile.
 - Stationary W_k[p, m] = (weight*mask)[112k + m, 112k - 8 + p]
   ([128, 112] per block, 1.06MB total). Bias is fused into the
   PSUM->SBUF drain (alternating ScalarE activation / VectorE
   tensor_scalar). Device writes out^T blocks [112, 2048]; host
   un-transposes and trims rows 4096..4143.
 - Input DMAs dispatch on the SP HWDGE; output DMAs on GpSimd (SWDGE)
   so input prefetch and drains never queue behind output writes.
"""

import os
import sys

for _p in ("/opt/trn_rl_repo", "/root/.axon_site/_ro/trn_rl_repo"):
    if os.path.isdir(_p) and _p not in sys.path:
        sys.path.append(_p)

import numpy as np
import ml_dtypes

import concourse.bacc as bacc
import concourse.mybir as mybir
from concourse.bass_utils import run_bass_kernel_spmd
from concourse.tile import TileContext

N_CORES = 8
N_TOK = 16384
NF = 4096
BAND = 8
TPC = N_TOK // N_CORES          # tokens per core (2048)
OB = 112                        # output rows per block (tile stride)
NB = (NF + OB - 1) // OB        # 37 blocks
NQ = NB // 2                    # 18 tile pairs (+1 tail tile)
CC = TPC // 512                 # token chunks of 512 per core (4)

COMPUTE_DT = os.environ.get("BAND_COMPUTE_DT", "bfloat16")
OUT_DT = os.environ.get("BAND_OUT_DT", "bfloat16")
XQ_BUFS = int(os.environ.get("BAND_XQ_BUFS", "6"))
O_BUFS = int(os.environ.get("BAND_O_BUFS", "8"))

LAST_RESULT = None  # BassKernelResults of the most recent run (for test.py)

_cache = {}


def _np_dt(name):
    return ml_dtypes.bfloat16 if name == "bfloat16" else np.float32


def _build(compute_dt: str, out_dt: str):
    cdt = getattr(mybir.dt, compute_dt)
    odt = getattr(mybir.dt, out_dt)
    f32 = mybir.dt.float32
    nc = bacc.Bacc("TRN2", target_bir_lowering=False, debug=False,
                   num_devices=N_CORES)
    # xq[q, p, kk*TPC + n] = x[n, 112*(2q+kk) - 8 + p]  (overlapped tiles)
    XQ = nc.dram_tensor("xt", [NQ, 128, 2 * TPC], cdt,
                        kind="ExternalInput").ap()
    XL = nc.dram_tensor("xl", [128, TPC], cdt, kind="ExternalInput").ap()
    WS = nc.dram_tensor("ws", [128, NB * OB], cdt,
                        kind="ExternalInput").ap()
    BM = nc.dram_tensor("bm", [128, NB], f32, kind="ExternalInput").ap()
    # ou[u, p, kk*TPC + n] = out^T[112*(2u+kk) + p, n] for p < 112
    OU = nc.dram_tensor("out", [NQ, OB, 2 * TPC], odt,
                        kind="ExternalOutput").ap()
    OUL = nc.dram_tensor("outl", [OB, TPC], odt,
                         kind="ExternalOutput").ap()

    ident = mybir.ActivationFunctionType.Identity
    add = mybir.AluOpType.add
    WH = NB * OB // 2              # ws half width (2072)

    with TileContext(nc) as tc:
        with (
            tc.tile_pool(name="bp", bufs=1) as bp,
            tc.tile_pool(name="xp", bufs=XQ_BUFS) as xp,
            tc.tile_pool(name="op", bufs=O_BUFS) as op,
            tc.tile_pool(name="pp", bufs=4, space="PSUM") as pp,
        ):
            bias_sb = bp.tile([128, NB], f32)
            ws_sb = bp.tile([128, NB * OB], cdt)
            xl_sb = bp.tile([128, TPC], cdt)

            xq_sb = {}

            def load_xq(q):
                t = xp.tile([128, 2 * TPC], cdt, tag="xq")
                nc.sync.dma_start(out=t[:], in_=XQ[q])
                xq_sb[q] = t

            def x_slice(k, c):
                if k == NB - 1:
                    return xl_sb[:, 512 * c:512 * c + 512]
                t = xq_sb[k // 2]
                base = (k % 2) * TPC + 512 * c
                return t[:, base:base + 512]

            # The first dispatched DMAs slip out before the trace-start
            # engine barrier, which then stalls dispatch ~2.7us twice —
            # lead with big x pair loads so those windows carry useful
            # bytes. Prefetch stays shallow (4 pairs + lookahead 4): if
            # reads race ahead of the drain pipeline, the run ends in a
            # sub-wall write-only tail.
            load_xq(0)
            # second lead pair dispatches on the Activation HWDGE: each
            # engine's stream can hoist one DMA ahead of the trace-start
            # rendezvous, so both barrier stalls carry useful bytes
            t1 = xp.tile([128, 2 * TPC], cdt, tag="xq")
            nc.scalar.dma_start(out=t1[:], in_=XQ[1])
            xq_sb[1] = t1
            nc.sync.dma_start(out=bias_sb[:], in_=BM[:])
            nc.sync.dma_start(out=ws_sb[:, 0:WH], in_=WS[:, 0:WH])
            load_xq(2)
            nc.sync.dma_start(out=ws_sb[:, WH:2 * WH], in_=WS[:, WH:2 * WH])
            load_xq(3)

            for k in range(NB):
                if k % 2 == 0 and k // 2 + 4 < NQ:
                    load_xq(k // 2 + 4)
                if k == 26:
                    nc.sync.dma_start(out=xl_sb[:], in_=XL[:])
                if k % 2 == 0:
                    ok2 = op.tile([OB, 2 * TPC], odt, tag="o")
                ob = (k % 2) * TPC
                stat = ws_sb[:, OB * k:OB * k + OB]
                bsl = bias_sb[0:OB, k:k + 1]
                for h in range(CC // 2):
                    # two chunks share a 2-bank PSUM tile -> one wide drain
                    p = pp.tile([OB, 1024], f32, tag="ps")
                    for cc in range(2):
                        c = 2 * h + cc
                        nc.tensor.matmul(p[:, 512 * cc:512 * cc + 512], stat,
                                         x_slice(k, c), start=True, stop=True)
                    osl = ok2[:, ob + 1024 * h:ob + 1024 * h + 1024]
                    if (k + h) % 2 == 0:
                        nc.scalar.activation(osl, p[:], ident, bias=bsl)
                    else:
                        nc.vector.tensor_scalar(osl, p[:], bsl, None, op0=add)
                if k == NB - 1:
                    nc.sync.dma_start(out=OUL[:], in_=ok2[:, 0:TPC])
                elif k % 2 == 1:
                    nc.sync.dma_start(out=OU[k // 2], in_=ok2[:])
    nc.finalize()
    return nc


def _get_nc(compute_dt, out_dt):
    key = (compute_dt, out_dt)
    if key not in _cache:
        _cache[key] = _build(compute_dt, out_dt)
    return _cache[key]


def kernel(x, weight, bias, mask):
    global LAST_RESULT
    x = np.asarray(x, dtype=np.float32)
    weight = np.asarray(weight, dtype=np.float32)
    bias = np.asarray(bias, dtype=np.float32)
    mask = np.asarray(mask, dtype=np.float32)

    cnp = _np_dt(COMPUTE_DT)
    wm = weight * mask                      # [O, I]

    # W_k[p, m] = wm[112k + m, 112k - 8 + p]  (zero off the edges)
    ws = np.zeros((128, NB * OB), dtype=np.float32)
    bm = np.zeros((128, NB), dtype=np.float32)
    for k in range(NB):
        mv = min(OB, NF - OB * k)
        lo, hi = OB * k - 8, min(OB * k + 120, NF)
        vlo = max(lo, 0)
        ws[vlo - lo:hi - lo, OB * k:OB * k + mv] = \
            wm[OB * k:OB * k + mv, vlo:hi].T
        bm[0:mv, k] = bias[OB * k:OB * k + mv]
    ws = np.ascontiguousarray(ws.astype(cnp))

    in_maps = []
    for ci in range(N_CORES):
        xs = x[TPC * ci:TPC * (ci + 1)]               # [TPC, NF]
        # xt_pad[112k + p, n] = x[n, 112k - 8 + p] (8 pad rows in front,
        # tail pad so 37 tiles of 128 fit)
        xt_pad = np.zeros((OB * (NB - 1) + 128, TPC), dtype=cnp)
        xt_pad[8:8 + NF] = xs.T
        idx = (OB * np.arange(NB - 1)[:, None, None]
               + np.arange(128)[None, :, None])
        tiles = xt_pad[idx[:, :, 0]]                  # [NB-1, 128, TPC]
        xq = (tiles.reshape(NQ, 2, 128, TPC).transpose(0, 2, 1, 3))
        xq = np.ascontiguousarray(xq).reshape(NQ, 128, 2 * TPC)
        xl = np.ascontiguousarray(xt_pad[OB * (NB - 1):OB * (NB - 1) + 128])
        in_maps.append({"xt": xq, "xl": xl, "ws": ws, "bm": bm})

    nc = _get_nc(COMPUTE_DT, OUT_DT)
    LAST_RESULT = run_bass_kernel_spmd(nc, in_maps, list(range(N_CORES)))

    out = np.empty((N_TOK, NF), dtype=np.float32)
    for ci in range(N_CORES):
        ou = np.asarray(LAST_RESULT.results[ci]["out"], dtype=np.float32)
        oul = np.asarray(LAST_RESULT.results[ci]["outl"], dtype=np.float32)
        ot = np.concatenate([
            ou.reshape(NQ, OB, 2, TPC).transpose(0, 2, 1, 3)
              .reshape((NB - 1) * OB, TPC),
            oul.reshape(OB, TPC)])
        out[TPC * ci:TPC * (ci + 1)] = ot[:NF].T
    return out


# revision 4
# speedup vs baseline: 1.1165x; 1.0283x over previous
"""BandLinear kernel for 8 TRN2 NeuronCores — final (overlapped 112-stride tiles, two-bank
PSUM drains, paired 8KB-line output DMAs, co-scheduled read/write
streams, barrier-gap-filling lead loads).

out[n, o] = sum_i x[n, i] * (weight * mask)[o, i] + bias[o]
with a +-8 band mask, x: [16384, 4096] f32.

Strategy (data-parallel over tokens, 2048 tokens/core):
 - Host pre-transposes each core's x shard into OVERLAPPED tiles of 128
   in-feature rows at stride 112: tile_k[p, n] = x[n, 112k - 8 + p]
   (zero-padded off the edges). A 112-row output block k (rows
   112k..112k+111) has its whole +-8 band inside tile k, so it needs
   exactly ONE K=128 matmul per token chunk — no accumulation couplings,
   no PE row-group switches (those flush the PE pipeline and were worth
   ~40us), at the cost of shipping 14% extra x bytes.
 - 37 tiles cover 4096 rows; tiles ride in pairs ([128, 2*2048] DMAs,
   8KB contiguous per-partition lines) + one single tail tile.
 - Stationary W_k[p, m] = (weight*mask)[112k + m, 112k - 8 + p]
   ([128, 112] per block, 1.06MB total). Bias is fused into the
   PSUM->SBUF drain (alternating ScalarE activation / VectorE
   tensor_scalar). Device writes out^T blocks [112, 2048]; host
   un-transposes and trims rows 4096..4143.
 - Input DMAs dispatch on the SP HWDGE; output DMAs on GpSimd (SWDGE)
   so input prefetch and drains never queue behind output writes.
"""

import os
import sys

for _p in ("/opt/trn_rl_repo", "/root/.axon_site/_ro/trn_rl_repo"):
    if os.path.isdir(_p) and _p not in sys.path:
        sys.path.append(_p)

import numpy as np
import ml_dtypes

import concourse.bacc as bacc
import concourse.mybir as mybir
from concourse.bass_utils import run_bass_kernel_spmd
from concourse.tile import TileContext

N_CORES = 8
N_TOK = 16384
NF = 4096
BAND = 8
TPC = N_TOK // N_CORES          # tokens per core (2048)
OB = 112                        # output rows per block (tile stride)
NB = (NF + OB - 1) // OB        # 37 blocks
NQ = NB // 2                    # 18 tile pairs (+1 tail tile)
CC = TPC // 512                 # token chunks of 512 per core (4)

COMPUTE_DT = os.environ.get("BAND_COMPUTE_DT", "bfloat16")
OUT_DT = os.environ.get("BAND_OUT_DT", "bfloat16")
XQ_BUFS = int(os.environ.get("BAND_XQ_BUFS", "6"))
O_BUFS = int(os.environ.get("BAND_O_BUFS", "8"))

LAST_RESULT = None  # BassKernelResults of the most recent run (for test.py)

_cache = {}


def _np_dt(name):
    return ml_dtypes.bfloat16 if name == "bfloat16" else np.float32


def _build(compute_dt: str, out_dt: str):
    cdt = getattr(mybir.dt, compute_dt)
    odt = getattr(mybir.dt, out_dt)
    f32 = mybir.dt.float32
    nc = bacc.Bacc("TRN2", target_bir_lowering=False, debug=False,
                   num_devices=N_CORES)
    # xq[q, p, kk*TPC + n] = x[n, 112*(2q+kk) - 8 + p]  (overlapped tiles)
    XQ = nc.dram_tensor("xt", [NQ, 128, 2 * TPC], cdt,
                        kind="ExternalInput").ap()
    XL = nc.dram_tensor("xl", [128, TPC], cdt, kind="ExternalInput").ap()
    WS = nc.dram_tensor("ws", [128, NB * OB], cdt,
                        kind="ExternalInput").ap()
    BM = nc.dram_tensor("bm", [128, NB], f32, kind="ExternalInput").ap()
    # ou[u, p, kk*TPC + n] = out^T[112*(2u+kk) + p, n] for p < 112
    OU = nc.dram_tensor("out", [NQ, OB, 2 * TPC], odt,
                        kind="ExternalOutput").ap()
    # block 36 only has 64 valid output rows (4032..4095) — ship just those
    OUL = nc.dram_tensor("outl", [64, TPC], odt,
                         kind="ExternalOutput").ap()

    ident = mybir.ActivationFunctionType.Identity
    add = mybir.AluOpType.add
    WH = NB * OB // 2              # ws half width (2072)

    with TileContext(nc) as tc:
        with (
            tc.tile_pool(name="bp", bufs=1) as bp,
            tc.tile_pool(name="xp", bufs=XQ_BUFS) as xp,
            tc.tile_pool(name="op", bufs=O_BUFS) as op,
            tc.tile_pool(name="pp", bufs=4, space="PSUM") as pp,
        ):
            bias_sb = bp.tile([128, NB], f32)
            ws_sb = bp.tile([128, NB * OB], cdt)
            xl_sb = bp.tile([128, TPC], cdt)

            xq_sb = {}

            def load_xq(q):
                t = xp.tile([128, 2 * TPC], cdt, tag="xq")
                nc.sync.dma_start(out=t[:], in_=XQ[q])
                xq_sb[q] = t

            def x_slice(k, c):
                if k == NB - 1:
                    return xl_sb[:, 512 * c:512 * c + 512]
                t = xq_sb[k // 2]
                base = (k % 2) * TPC + 512 * c
                return t[:, base:base + 512]

            # The first dispatched DMAs slip out before the trace-start
            # engine barrier, which then stalls dispatch ~2.7us twice —
            # lead with big x pair loads so those windows carry useful
            # bytes. Prefetch stays shallow (4 pairs + lookahead 4): if
            # reads race ahead of the drain pipeline, the run ends in a
            # sub-wall write-only tail.
            load_xq(0)
            # second lead pair dispatches on the Activation HWDGE: each
            # engine's stream can hoist one DMA ahead of the trace-start
            # rendezvous, so both barrier stalls carry useful bytes
            t1 = xp.tile([128, 2 * TPC], cdt, tag="xq")
            nc.scalar.dma_start(out=t1[:], in_=XQ[1])
            xq_sb[1] = t1
            nc.sync.dma_start(out=bias_sb[:], in_=BM[:])
            nc.sync.dma_start(out=ws_sb[:, 0:WH], in_=WS[:, 0:WH])
            load_xq(2)
            nc.sync.dma_start(out=ws_sb[:, WH:2 * WH], in_=WS[:, WH:2 * WH])
            load_xq(3)

            for k in range(NB):
                if k % 2 == 0 and k // 2 + 4 < NQ:
                    load_xq(k // 2 + 4)
                if k == 26:
                    nc.sync.dma_start(out=xl_sb[:], in_=XL[:])
                if k % 2 == 0:
                    ok2 = op.tile([OB, 2 * TPC], odt, tag="o")
                ob = (k % 2) * TPC
                stat = ws_sb[:, OB * k:OB * k + OB]
                bsl = bias_sb[0:OB, k:k + 1]
                for h in range(CC // 2):
                    # two chunks share a 2-bank PSUM tile -> one wide drain
                    p = pp.tile([OB, 1024], f32, tag="ps")
                    for cc in range(2):
                        c = 2 * h + cc
                        nc.tensor.matmul(p[:, 512 * cc:512 * cc + 512], stat,
                                         x_slice(k, c), start=True, stop=True)
                    osl = ok2[:, ob + 1024 * h:ob + 1024 * h + 1024]
                    if (k + h) % 2 == 0:
                        nc.scalar.activation(osl, p[:], ident, bias=bsl)
                    else:
                        nc.vector.tensor_scalar(osl, p[:], bsl, None, op0=add)
                if k == NB - 1:
                    nc.sync.dma_start(out=OUL[:], in_=ok2[0:64, 0:TPC])
                elif k % 2 == 1:
                    nc.sync.dma_start(out=OU[k // 2], in_=ok2[:])
    nc.finalize()
    return nc


def _get_nc(compute_dt, out_dt):
    key = (compute_dt, out_dt)
    if key not in _cache:
        _cache[key] = _build(compute_dt, out_dt)
    return _cache[key]


def kernel(x, weight, bias, mask):
    global LAST_RESULT
    x = np.asarray(x, dtype=np.float32)
    weight = np.asarray(weight, dtype=np.float32)
    bias = np.asarray(bias, dtype=np.float32)
    mask = np.asarray(mask, dtype=np.float32)

    cnp = _np_dt(COMPUTE_DT)
    wm = weight * mask                      # [O, I]

    # W_k[p, m] = wm[112k + m, 112k - 8 + p]  (zero off the edges)
    ws = np.zeros((128, NB * OB), dtype=np.float32)
    bm = np.zeros((128, NB), dtype=np.float32)
    for k in range(NB):
        mv = min(OB, NF - OB * k)
        lo, hi = OB * k - 8, min(OB * k + 120, NF)
        vlo = max(lo, 0)
        ws[vlo - lo:hi - lo, OB * k:OB * k + mv] = \
            wm[OB * k:OB * k + mv, vlo:hi].T
        bm[0:mv, k] = bias[OB * k:OB * k + mv]
    ws = np.ascontiguousarray(ws.astype(cnp))

    in_maps = []
    for ci in range(N_CORES):
        xs = x[TPC * ci:TPC * (ci + 1)]               # [TPC, NF]
        # xt_pad[112k + p, n] = x[n, 112k - 8 + p] (8 pad rows in front,
        # tail pad so 37 tiles of 128 fit)
        xt_pad = np.zeros((OB * (NB - 1) + 128, TPC), dtype=cnp)
        xt_pad[8:8 + NF] = xs.T
        idx = (OB * np.arange(NB - 1)[:, None, None]
               + np.arange(128)[None, :, None])
        tiles = xt_pad[idx[:, :, 0]]                  # [NB-1, 128, TPC]
        xq = (tiles.reshape(NQ, 2, 128, TPC).transpose(0, 2, 1, 3))
        xq = np.ascontiguousarray(xq).reshape(NQ, 128, 2 * TPC)
        xl = np.ascontiguousarray(xt_pad[OB * (NB - 1):OB * (NB - 1) + 128])
        in_maps.append({"xt": xq, "xl": xl, "ws": ws, "bm": bm})

    nc = _get_nc(COMPUTE_DT, OUT_DT)
    LAST_RESULT = run_bass_kernel_spmd(nc, in_maps, list(range(N_CORES)))

    out = np.empty((N_TOK, NF), dtype=np.float32)
    for ci in range(N_CORES):
        ou = np.asarray(LAST_RESULT.results[ci]["out"], dtype=np.float32)
        oul = np.asarray(LAST_RESULT.results[ci]["outl"], dtype=np.float32)
        ot = np.concatenate([
            ou.reshape(NQ, OB, 2, TPC).transpose(0, 2, 1, 3)
              .reshape((NB - 1) * OB, TPC),
            oul.reshape(64, TPC)])
        out[TPC * ci:TPC * (ci + 1)] = ot.T
    return out
